# revision 1
# baseline (speedup 1.0000x reference)
"""Masked dot-product attention on 8 Trainium2 NeuronCores (Bass/Tile).

Problem: B=16, LQ=LK=2048, D=128 fp32; per-batch key valid_lens mask.
Sharding: 64 (batch, 512-query-block) units bin-packed into 8 slots x 8
cores by per-batch valid k-tile count, so every core runs an identical
(SPMD) program while skipping masked-out key tiles entirely.

Per unit, on device (scores kept transposed so no probability transpose
is ever needed):
  S^T[k, q]  = KT_tile.T @ QT_block       (fp16 matmuls, N=512, full rate)
  W^T        = exp(S^T/sqrt(D))           (ScalarE, 2-tile groups)
  rowsum[q] += ones.T @ W^T               (PE, M=1, PSUM accumulate; DVE
                                           pair-adds so one matmul covers
                                           up to 4 k-tiles)
  O^T[d, q] += V_tile.T @ W^T             (PE, fp16, PSUM accumulate)
then the unnormalized O^T (fp16) and fp32 rowsums are DMA'd out and the
host normalizes. Masking needs NO on-device work at all: K columns and
V rows at/beyond valid_len are zero-padded, so each invalid position
contributes exactly w = exp(0) = 1 to the rowsum and 0 to the numerator;
the host subtracts the known count (u_s*128 - valid_len) from the fp32
rowsum before dividing. The bin-packing keeps slot-mates of similar
valid_len, so the correction stays small relative to the true rowsum
and costs ~1e-5 relative accuracy.

Keeping the output transposed kills the on-chip transpose (PE columns,
a PSUM bank, and a DVE copy per slot) and shortens the epilogue to a
PE-free chain that hides under the next slot's compute. PSUM budget:
S^T 2x2 + O^T 2 + rowsum 2 = 8 banks, double-buffered throughout.
"""

import math

import ml_dtypes
import numpy as np

import concourse.bass as bass
import concourse.mybir as mybir
import concourse.tile as tile
from concourse import bacc
from concourse.bass_utils import run_bass_kernel_spmd

B, LQ, LK, D = 16, 2048, 2048, 128
N_CORES = 8
QB = 512          # query block (one unit) = QB rows of Q
N_SLOTS = (LQ // QB) * B // N_CORES   # 8 slots per core
KT = 128          # key tile
F32 = mybir.dt.float32
FP16 = mybir.dt.float16
SCALE = 1.0 / math.sqrt(D)


def _plan(valid_lens):
    """Assign 64 (batch, qblock) units to an 8x8 (slot, core) grid.

    Returns (slot_units, slot_ntiles) where
      slot_units[s][c] = (batch, qblock) handled by core c in slot s
      slot_ntiles[s]   = k-tiles processed in slot s (max over cores)
    """
    vl = np.asarray(valid_lens).astype(np.int64)
    ktiles = np.maximum(1, np.ceil(vl / KT).astype(np.int64))
    units = [(int(b), j) for b in range(B) for j in range(LQ // QB)]
    units.sort(key=lambda u: -ktiles[u[0]])
    slot_units, slot_ntiles = [], []
    for s in range(N_SLOTS):
        chunk = units[s * N_CORES:(s + 1) * N_CORES]
        slot_units.append(chunk)
        slot_ntiles.append(int(max(ktiles[b] for b, _ in chunk)))
    # the smallest slot first (compute starts after only a couple of
    # k-tiles land), then descending, so small tail slots never stack
    # their epilogue chains at the kernel tail.
    order = [N_SLOTS - 1, 0, 1, 2, 4, 5, 6, 3]
    slot_units = [slot_units[s] for s in order]
    slot_ntiles = [slot_ntiles[s] for s in order]
    return slot_units, slot_ntiles


def _pack_inputs(queries, keys, values, valid_lens, slot_units, slot_ntiles):
    """Build per-core packed input arrays (host-side numpy).

    K columns / V rows at-or-beyond valid_len are zero (including whole
    padding tiles): scores there are 0, so exp contributes exactly 1.0
    to the rowsum (corrected on the host) and nothing to the numerator.
    """
    vl = np.asarray(valid_lens).astype(np.int64)
    qt = np.ascontiguousarray(np.transpose(queries, (0, 2, 1)))   # [B, D, LQ]
    kt = np.ascontiguousarray(np.transpose(keys, (0, 2, 1)))      # [B, D, LK]
    u_total = sum(slot_ntiles)
    in_maps = []
    for c in range(N_CORES):
        qt_p = np.zeros((N_SLOTS, D, QB), np.float16)
        # per tile: [:, :KT] = K^T tile (row=d), [:, KT:] = V tile (row=k);
        # one combined tensor so each preload chunk is a single DMA with
        # 512B-contiguous descriptors
        kv_p = np.zeros((u_total, 128, KT + D), np.float16)
        off = 0
        for s in range(N_SLOTS):
            b, j = slot_units[s][c]
            qt_p[s] = qt[b, :, j * QB:(j + 1) * QB]
            nv = int(min(slot_ntiles[s] * KT, vl[b]))
            nt = (nv + KT - 1) // KT
            for t in range(nt):
                k0 = t * KT
                n = min(KT, nv - k0)
                kv_p[off + t, :, :n] = kt[b, :, k0:k0 + n]
                kv_p[off + t, :n, KT:] = values[b, k0:k0 + n, :]
            off += slot_ntiles[s]
        in_maps.append({"qt": qt_p, "kv": kv_p})
    return in_maps, u_total


def build_kernel(slot_ntiles, u_total, reps=1):
    nc = bacc.Bacc(None, target_bir_lowering=False, debug=True)
    qt_d = nc.dram_tensor("qt", [N_SLOTS, D, QB], FP16, kind="ExternalInput")
    kv_d = nc.dram_tensor(
        "kv", [u_total, 128, KT + D], FP16, kind="ExternalInput")
    out_d = nc.dram_tensor("out", [N_SLOTS, D, QB], FP16, kind="ExternalOutput")
    rs_d = nc.dram_tensor("rsum", [N_SLOTS, 1, QB], F32, kind="ExternalOutput")

    G = 2  # k-tiles per exp group (PSUM banks: st 2x2 + ot 2 + rs 2 = 8)

    with tile.TileContext(nc) as tc:
        with (
            tc.tile_pool(name="const", bufs=1) as const,
            tc.tile_pool(name="wt_pool", bufs=8) as wt_pool,
            tc.tile_pool(name="ws_pool", bufs=8) as ws_pool,
            tc.tile_pool(name="ocopy_pool", bufs=2) as ocopy_pool,
            tc.tile_pool(name="rscopy_pool", bufs=2) as rscopy_pool,
            tc.tile_pool(name="st_psum", bufs=3, space="PSUM") as st_psum,
            tc.tile_pool(name="ot_psum", bufs=1, space="PSUM") as ot_psum,
            tc.tile_pool(name="rs_psum", bufs=1, space="PSUM") as rs_psum,
        ):
            ones = const.tile([128, 1], FP16)
            nc.vector.memset(ones, 1.0)
            # Preload: k/v stream through the low-latency SP HWDGE queue
            # in just-in-time order (minimal slot-0 working set first);
            # the later qt slots ride the otherwise-idle Pool SWDGE queue
            # so they never block it.
            qt_all = const.tile([128, N_SLOTS, QB], FP16)
            kv_all = const.tile([128, u_total, KT + D], FP16)

            def load_kv(lo, hi):
                if hi > lo:
                    nc.sync.dma_start(
                        out=kv_all[:, lo:hi, :],
                        in_=kv_d[lo:hi].rearrange("u p c -> p u c"))

            nc.scalar.dma_start(
                out=qt_all[:, 0:1, :],
                in_=qt_d[0:1].rearrange("s d q -> d s q"))
            load_kv(0, 2)
            load_kv(2, 6)
            load_kv(6, min(11, u_total))
            load_kv(11, min(16, u_total))
            nc.gpsimd.dma_start(
                out=qt_all[:, 1:4, :],
                in_=qt_d[1:4].rearrange("s d q -> d s q"))
            bnds = [round(16 + i * (u_total - 16) / 4) for i in range(5)]
            load_kv(bnds[0], bnds[1])
            load_kv(bnds[1], bnds[2])
            nc.gpsimd.dma_start(
                out=qt_all[:, 4:N_SLOTS, :],
                in_=qt_d[4:N_SLOTS].rearrange("s d q -> d s q"))
            load_kv(bnds[2], bnds[3])
            load_kv(bnds[3], bnds[4])

            # PE warm-up: tiny dependency-free matmuls keep the tensor
            # engine's activity monitor busy while the preload DMAs are in
            # flight, so the first real QKs run at the warm clock rate.
            warm_st = st_psum.tile([128, G, QB], F32, tag="st")
            for _ in range(120):
                nc.tensor.matmul(warm_st[0:1, 0, 0:1], ones, ones,
                                 start=True, stop=True)

            # Global software pipeline: the QK matmuls run one group AHEAD
            # of the exp/rowsum/PV stream (across slot boundaries too), so
            # the in-order PE queue always has the next group's scores in
            # flight while ScalarE works and the PV tail of a slot never
            # starves the next slot's first exp.
            n_slots = len(slot_ntiles)
            flat = []     # (slot, group start, group size, slot group idx)
            for _rep in range(reps):
                off = 0
                for s in range(n_slots):
                    u_s = slot_ntiles[s]
                    groups = [(t0, min(G, u_s - t0))
                              for t0 in range(0, u_s, G)]
                    for gi, (g, gsz) in enumerate(groups):
                        flat.append((_rep, s, off, g, gsz, gi, len(groups)))
                    off += u_s

            state = {}    # per (rep, slot) live tiles
            sts = {}      # in-flight score tiles keyed by flat index
            pending_epi = []

            def emit_qk(i):
                _rep, s, off, g, gsz, gi, ng = flat[i]
                st = st_psum.tile([128, G, QB], F32)
                for tt in range(gsz):
                    t = g + tt
                    nc.tensor.matmul(
                        st[:, tt, :], kv_all[:, off + t, :KT],
                        qt_all[:, s, :], start=True, stop=True,
                    )
                sts[i] = st

            emit_qk(0)
            emit_qk(1)
            for i in range(len(flat)):
                _rep, s, off, g, gsz, gi, ng = flat[i]
                u_s = slot_ntiles[s]
                if gi == 0:
                    ot = ot_psum.tile([128, QB], F32, tag="ot")  # O^T [d, q]
                    rs = rs_psum.tile([1, QB], F32, tag="rs")    # rowsum
                    state[(_rep, s)] = (ot, rs, [None, 0])
                ot, rs, rs_state = state[(_rep, s)]
                st = sts.pop(i)
                wt = wt_pool.tile([128, G, QB], FP16)
                nc.scalar.activation(
                    wt[:, :gsz, :], st[:, :gsz, :],
                    mybir.ActivationFunctionType.Exp, scale=SCALE,
                )
                if i + 2 < len(flat):
                    emit_qk(i + 2)
                if gi == 0 and pending_epi:
                    # previous slot's epilogue (no PE instructions) goes
                    # here, hidden under this slot's compute
                    pending_epi.pop(0)()
                # rowsum is linear in the k-tiles: DVE sum-tree (pair
                # tiles, then pair groups) so one PE rowsum matmul covers
                # up to 4 k-tiles
                if gsz == 2:
                    ws = ws_pool.tile([128, QB], FP16)
                    nc.vector.tensor_add(ws, wt[:, 0, :], wt[:, 1, :])
                    rs_src = ws
                else:
                    rs_src = wt[:, 0, :]
                # in the final slot, one rs matmul per group: a shorter
                # DVE chain ahead of the kernel-tail rowsum
                last_slot = _rep == reps - 1 and s == n_slots - 1
                if rs_state[0] is None and gi < ng - 1 and not last_slot:
                    rs_state[0] = rs_src
                else:
                    if rs_state[0] is not None:
                        ws2 = ws_pool.tile([128, QB], FP16)
                        nc.vector.tensor_add(ws2, rs_state[0], rs_src)
                        rs_src = ws2
                        rs_state[0] = None
                    nc.tensor.matmul(
                        rs, ones, rs_src,
                        start=(rs_state[1] == 0),
                        stop=(gi == ng - 1),
                    )
                    rs_state[1] += 1
                for tt in range(gsz):
                    t = g + tt
                    nc.tensor.matmul(
                        ot, kv_all[:, off + t, KT:], wt[:, tt, :],
                        start=(t == 0), stop=(t == u_s - 1),
                    )
                if gi == ng - 1:
                    def _epilogue(s=s, ot=ot, rs=rs, tail=False):
                        # normalization happens host-side: just evict the
                        # unnormalized O^T (fp16) and the fp32 rowsums.
                        # Flush epilogues (kernel tail) split across the
                        # then-idle ACT engine/queue to shorten the chain.
                        ocopy = ocopy_pool.tile([128, QB], FP16)
                        rscopy = rscopy_pool.tile([1, QB], F32)
                        if tail:
                            nc.scalar.copy(ocopy, ot)
                            nc.vector.tensor_copy(rscopy, rs)
                            nc.scalar.dma_start(out=out_d[s], in_=ocopy)
                            nc.sync.dma_start(out=rs_d[s], in_=rscopy)
                        else:
                            nc.vector.tensor_copy(ocopy, ot)
                            nc.vector.tensor_copy(rscopy, rs)
                            nc.sync.dma_start(out=out_d[s], in_=ocopy)
                            nc.sync.dma_start(out=rs_d[s], in_=rscopy)
                    pending_epi.append(_epilogue)
                    del state[(_rep, s)]
            for fn in pending_epi:
                fn(tail=True)
    nc.finalize()
    return nc


def kernel(queries, keys, values, valid_lens):
    queries = np.ascontiguousarray(np.asarray(queries, dtype=np.float32))
    keys = np.ascontiguousarray(np.asarray(keys, dtype=np.float32))
    values = np.ascontiguousarray(np.asarray(values, dtype=np.float32))
    assert queries.shape == (B, LQ, D), queries.shape
    assert keys.shape == (B, LK, D), keys.shape
    assert values.shape == (B, LK, D), values.shape

    slot_units, slot_ntiles = _plan(valid_lens)
    in_maps, u_total = _pack_inputs(
        queries, keys, values, valid_lens, slot_units, slot_ntiles)
    nc = build_kernel(slot_ntiles, u_total)
    res = None
    last_exc = None
    for attempt in range(3):
        try:
            res = run_bass_kernel_spmd(nc, in_maps, list(range(N_CORES)))
            break
        except Exception as exc:  # transient NRT/axon failures
            last_exc = exc
            try:
                import jax
                jax.clear_caches()
            except Exception:
                pass
    if res is None:
        raise last_exc

    vl = np.asarray(valid_lens).astype(np.int64)
    out = np.empty((B, LQ, D), np.float32)
    for c in range(N_CORES):
        o = res.results[c]["out"]        # [N_SLOTS, D, QB] fp16, unnormalized
        rsum = res.results[c]["rsum"]    # [N_SLOTS, 1, QB] f32 softmax denom
        for s in range(N_SLOTS):
            b, j = slot_units[s][c]
            corr = float(slot_ntiles[s] * KT - vl[b])
            out[b, j * QB:(j + 1) * QB, :] = (
                o[s].astype(np.float32) / (rsum[s] - corr)).T
    return out

